# revision 18
# baseline (speedup 1.0000x reference)
"""Trainium2 Bass kernel for nn_DiscriminativeLoss.

Shapes (hardcoded): embedded [16, 4096, 32] f32, masks [16, 4096, 64] f32,
size [16] i32.  Data-parallel over batch: 2 samples per NeuronCore x 8 cores.

Per-sample math (fp8 masks x fp16 operands, fp32 PSUM accumulation):
  MM-A   SUMS[k, 0:33]  = sum_n m[n,k] * [e | 1][n, :]     (centroid sums+counts)
  W  = [-2c | c2 | 1],  W2 = [c | 1 | c2]  where c = valid * sums / max(cnt,1)
  MM-B   CSEL[n, :] = m[n, :] @ W                           (per-point gather)
  d2o[n] = sum_j X[n,j]*CSEL[n,j],  X = [e | 1 | e2]        (= ||e_n - c_own||^2)
  jv2[n] = relu(sqrt(d2o) - 0.5)^2                          (L_v numerator terms)
  D2P    = T(W2)^T @ T(W) = -2 c.c' + c2[k] + c2[k']        (pair distances)
  hd2    = relu(3 - sqrt(max(D2P, 0) + pvbig))^2            (L_d numerator terms)
  rtv    = valid * sqrt(c2)                                 (L_r numerator terms)
A final ones-matmul reduces all numerator terms over partitions to [2, 132];
host does the remaining tiny column sums, denominators and the batch mean.

Masks ship as fp8 (0/1 exact) in two layouts (natural for MM-A stationary,
transposed for MM-B stationary w/ fast-weight-load), packed into one fp16
input tensor read back via a bitcast view.  A dummy sqrt at kernel start
prefetches the single ACT table set during the input-DMA window; ACT
evacuates MM-B's PSUM blocks to fp16 so the big elementwise multiplies run
in the DVE 2x packed mode, and the final partition reduction is one
ones-matmul whose [2, 132] output the host finishes summing.  Relies on
masks rows being one-hot (exactly what reference.setup_inputs produces).
"""

import numpy as np
import ml_dtypes

import concourse.bacc as bacc
import concourse.mybir as mybir
from concourse import tile
from concourse.bass_utils import run_bass_kernel_spmd
from concourse.mybir import ActivationFunctionType as Act, AluOpType as Op

B, N, K, E = 16, 4096, 64, 32
NCORES = 8
SPC = B // NCORES          # samples per core
J = N // 128               # 32 n-chunks of 128
CW = E + 2                 # 34: [e | 1 | e2]
DT = mybir.dt.float16
F8 = mybir.dt.float8e4
F32 = mybir.dt.float32
NPDT = np.float16
NPF8 = ml_dtypes.float8_e4m3

XEW = J * CW               # 1088 fp16 cols per sample of [e|1|e2]
MNW8 = J * K               # 2048 fp8 cols per sample of mask-natural
XEOFF = MNW8               # 2048 fp16 cols hold both samples' fp8 mn blocks
INAW = XEOFF + SPC * XEW   # 4224 fp16 cols
CSTW = 72
CS2W = 68
OUTW = 132

_CACHE = {}


def _build_nc():
    if "nc" in _CACHE:
        return _CACHE["nc"]
    nc = bacc.Bacc("TRN2", target_bir_lowering=False, debug=False)
    cst_d = nc.dram_tensor("cst", [128, CSTW], F32, kind="ExternalInput").ap()
    cs2_d = nc.dram_tensor("cs2", [128, CS2W], DT, kind="ExternalInput").ap()
    ina_d = nc.dram_tensor("ina", [128, INAW], DT, kind="ExternalInput").ap()
    mtt_d = nc.dram_tensor("mtt", [128, N], F8, kind="ExternalInput").ap()
    out_d = nc.dram_tensor("out", [2, OUTW], F32, kind="ExternalOutput").ap()

    # ---- pre-TileContext loads; the mtt DMA is issued in-context so it
    # trails the critical ina transfer in the queue. ----
    CST = nc.alloc_sbuf_tensor("cst_sb", [128, CSTW], F32).ap()
    CS2 = nc.alloc_sbuf_tensor("cs2_sb", [128, CS2W], DT).ap()
    INA = nc.alloc_sbuf_tensor("ina_sb", [128, INAW], DT).ap()
    sem_c = nc.alloc_semaphore("sem_c")
    sem_a = nc.alloc_semaphore("sem_a")
    nc.sync.dma_start(INA[:], ina_d[:]).then_inc(sem_a, 16)
    nc.sync.dma_start(CST[:], cst_d[:]).then_inc(sem_c, 16)
    nc.sync.dma_start(CS2[:], cs2_d[:]).then_inc(sem_c, 16)
    nc.tensor.wait_ge(sem_c, 32)
    nc.tensor.wait_ge(sem_a, 16)
    nc.vector.wait_ge(sem_c, 32)
    nc.scalar.wait_ge(sem_c, 32)

    INAF8 = INA.bitcast(F8)    # [128, 2*INAW]; fp8 mask-natural in cols 0:4096

    def mn(s, j):              # mask-natural chunk j of sample s  [128, 64] fp8
        return INAF8[:, s * MNW8 + j * K : s * MNW8 + (j + 1) * K]

    def xe(s, lo, hi):         # [e|1|e2] cols of sample s  (fp16)
        return INA[:, XEOFF + s * XEW + lo : XEOFF + s * XEW + hi]

    valid_c = CST[:, 0:1]
    vm2_c = CST[:, 1:2]        # -2 * valid
    b3_c = CST[:, 2:3]         # 3.0
    pvbig_c = CST[:, 4 : 4 + K]
    IDN = CS2[:, 0:64]
    ONES2 = CS2[:, 64:66]      # col0 = lower-half ones, col1 = upper-half ones

    with tile.TileContext(nc) as tc:
        with (
            tc.tile_pool(name="io", bufs=1) as io,
            tc.tile_pool(name="wk", bufs=2) as wk,
            tc.tile_pool(name="ps", bufs=1, space="PSUM") as ps,
        ):
            # dummy sqrt: triggers the single ACT table-set load (~1.3us)
            # during the input-DMA window instead of mid-kernel.  The mtt
            # DMA is issued on the ACT ring after it, so the critical ina
            # transfer gets the full HBM bandwidth first.
            MTT = io.tile([128, N], F8, tag="mtt")
            nc.sync.dma_start(MTT[:], mtt_d[:])
            warm_i = wk.tile([128, 1], F32, tag="warm_i")
            warm_o = wk.tile([128, 1], F32, tag="warm_o")
            nc.gpsimd.memset(warm_i[:], 4.0)
            nc.scalar.activation(warm_o[:], warm_i[:], Act.Sqrt)

            WST = wk.tile([128, CW], DT, tag="wst")    # [-2c | c2 | 1]
            W2 = wk.tile([128, CW], DT, tag="w2")      # [c | 1 | c2]
            FINSRC = wk.tile([128, OUTW], DT, tag="finsrc")
            nc.vector.memset(WST[:, 33:34], 1.0)
            nc.vector.memset(W2[:, 32:33], 1.0)
            nc.vector.memset(FINSRC[:, 129:132], 0.0)

            # ---- MM-A: both samples concurrently via column tiling ----
            SUMS = ps.tile([128, 64], F32, tag="sumsa")
            SPS = [SUMS[0:K], SUMS[K:128]]
            for j in range(J):
                for s in range(SPC):
                    nc.tensor.matmul(
                        SPS[s][:, 0:33], mn(s, j), xe(s, j * CW, j * CW + 33),
                        start=(j == 0), stop=(j == J - 1),
                        tile_position=(0, 64 * s),
                    )

            # ---- centroid factors, both samples at once ----
            D2O = wk.tile([128, 66], F32, tag="d2o")   # cols 0:64 d2o, 64 = c2
            cnt1 = wk.tile([128, 1], F32, tag="cnt1")
            nc.vector.tensor_scalar(cnt1[:], SUMS[:, 32:33], 1.0, None, Op.max)
            rec = wk.tile([128, 1], F32, tag="rec")
            nc.vector.reciprocal(rec[:], cnt1[:])
            recm2 = wk.tile([128, 1], F32, tag="recm2")
            nc.vector.tensor_scalar(recm2[:], rec[:], vm2_c, None, Op.mult)
            # c2 = 0.25 * recm2^2 * sum(sums^2)
            ssq = wk.tile([128, 1], F32, tag="ssq")
            sqj = wk.tile([128, 32], F32, tag="sqj")
            nc.scalar.activation(
                sqj[:], SUMS[:, 0:32], Act.Square, accum_out=ssq[:]
            )
            nc.scalar.activation(
                WST[:, 0:32], SUMS[:, 0:32], Act.Copy, bias=0.0, scale=recm2[:]
            )
            r2 = wk.tile([128, 1], F32, tag="r2")
            nc.vector.tensor_tensor(r2[:], recm2[:], recm2[:], Op.mult)
            nc.vector.scalar_tensor_tensor(
                D2O[:, 64:65], ssq[:], 0.25, r2[:], Op.mult, Op.mult
            )
            nc.vector.tensor_copy(WST[:, 32:33], D2O[:, 64:65])
            recp1 = wk.tile([128, 1], F32, tag="recp1")
            nc.vector.tensor_scalar(recp1[:], rec[:], valid_c, None, Op.mult)
            nc.scalar.activation(
                W2[:, 0:32], SUMS[:, 0:32], Act.Copy, bias=0.0, scale=recp1[:]
            )
            nc.vector.tensor_copy(W2[:, 33:34], D2O[:, 64:65])

            # ---- MM-B + per-point distances; samples on row-groups ----
            PBS = [None, None]
            for h in range(2):
                for s in range(SPC):
                    PB = ps.tile([128, 1024], F32, tag=f"pb{s}")
                    PBS[s] = PB
                    for i in range(16):
                        j = h * 16 + i
                        off = 512 * (i // 8) + CW * (i % 8)
                        nc.tensor.matmul(
                            PB[:, off : off + CW],
                            MTT[s * K : (s + 1) * K, j * 128 : (j + 1) * 128],
                            WST[s * K : (s + 1) * K, 0:CW],
                            start=True, stop=True,
                            tile_position=(64 * s, 0),
                        )
                for s in range(SPC):
                    PB = PBS[s]
                    EV = wk.tile([128, 2 * 8 * CW], DT, tag=f"ev{s}")
                    pb3 = PB[:].rearrange("p (b q) -> p b q", b=2)[:, :, 0 : 8 * CW]
                    ev3 = EV[:].rearrange("p (b q) -> p b q", b=2)
                    nc.scalar.activation(ev3, pb3, Act.Copy)
                    PR = wk.tile([128, 2 * 8 * CW], DT, tag="pr")
                    xe3 = xe(s, h * 16 * CW, (h + 1) * 16 * CW).rearrange(
                        "p (b q) -> p b q", b=2
                    )
                    pr3 = PR[:].rearrange("p (b q) -> p b q", b=2)
                    nc.vector.tensor_tensor(pr3, EV[:], xe(s, h * 16 * CW, (h + 1) * 16 * CW), Op.mult)
                    nc.vector.tensor_reduce(
                        D2O[:, s * 32 + h * 16 : s * 32 + h * 16 + 16],
                        PR[:].rearrange("p (j c) -> p j c", c=CW),
                        axis=mybir.AxisListType.X,
                        op=Op.add,
                    )

            # ---- L_d: pair distances from transposed W / W2 ----
            TWt = ps.tile([128, K], DT, tag="twt")
            LTt = ps.tile([128, K], DT, tag="ltt")
            for s in range(SPC):
                nc.tensor.transpose(
                    TWt[64 * s : 64 * s + CW, :],
                    WST[s * K : (s + 1) * K, 0:CW],
                    IDN[s * K : (s + 1) * K, :],
                    tile_position=(64 * s, 64 * s),
                )
                nc.tensor.transpose(
                    LTt[64 * s : 64 * s + CW, :],
                    W2[s * K : (s + 1) * K, 0:CW],
                    IDN[s * K : (s + 1) * K, :],
                    tile_position=(64 * s, 64 * s),
                )
            TW = wk.tile([128, K], DT, tag="tw")
            LT = wk.tile([128, K], DT, tag="lt")
            nc.scalar.activation(TW[:], TWt[:], Act.Copy)
            nc.scalar.activation(LT[:], LTt[:], Act.Copy)
            D2P = ps.tile([128, K], F32, tag="sumsa")
            for s in range(SPC):
                nc.tensor.matmul(
                    D2P[64 * s : 64 * s + 64, :],
                    LT[64 * s : 64 * s + CW, :],
                    TW[64 * s : 64 * s + CW, :],
                    start=True, stop=True,
                    tile_position=(64 * s, 64 * s),
                )
            DSm = wk.tile([128, K], F32, tag="dsm")
            nc.vector.scalar_tensor_tensor(
                DSm[:], D2P[:], 0.0, pvbig_c, Op.max, Op.add
            )
            NS = wk.tile([128, K], F32, tag="ns")
            nc.scalar.activation(NS[:], DSm[:], Act.Sqrt)
            HD = wk.tile([128, K], F32, tag="hd")
            nc.scalar.activation(HD[:], NS[:], Act.Relu, bias=b3_c, scale=-1.0)
            nc.gpsimd.tensor_tensor(FINSRC[:, 64:128], HD[:], HD[:], Op.mult)

            # ---- L_v + L_r tails ----
            DN65 = wk.tile([128, 65], F32, tag="dn65")
            nc.scalar.activation(DN65[:], D2O[:, 0:65], Act.Sqrt)
            HV = wk.tile([128, 64], DT, tag="hv")
            nc.vector.tensor_scalar(HV[:], DN65[:, 0:64], -0.5, 0.0, Op.add, Op.max)
            nc.vector.tensor_tensor(FINSRC[:, 0:64], HV[:], HV[:], Op.mult)
            nc.vector.tensor_scalar(
                FINSRC[:, 128:129], DN65[:, 64:65], valid_c, None, Op.mult
            )

            # ---- partition reduction: row 0 = lower half, row 1 = upper ----
            FIN = ps.tile([2, OUTW], F32, tag="twt")
            nc.tensor.matmul(FIN[:], ONES2, FINSRC[:], start=True, stop=True)
            FOUT = wk.tile([2, OUTW], F32, tag="fout")
            nc.vector.tensor_copy(FOUT[:], FIN[:])
            nc.sync.dma_start(out_d[:], FOUT[:])

    nc.compile()
    _CACHE["nc"] = nc
    return nc


def pack_inputs(embedded, masks, size):
    emb = np.asarray(embedded, dtype=np.float32)
    msk = np.asarray(masks, dtype=np.float32)
    sz = np.asarray(size).astype(np.int64)
    ar = np.arange(K)
    eye = np.eye(K, dtype=np.float32)
    in_maps, meta = [], []
    for c in range(NCORES):
        ina = np.zeros((128, INAW), NPDT)
        mn8 = np.zeros((128, 2 * MNW8), NPF8)
        mtt = np.zeros((128, N), NPF8)
        cst = np.zeros((128, CSTW), np.float32)
        cs2 = np.zeros((128, CS2W), NPDT)
        cst[:, 2] = 3.0
        cs2[0:K, 0:K] = eye
        cs2[K:128, 0:K] = eye
        cs2[0:K, 64] = 1.0
        cs2[K:128, 65] = 1.0
        for s in range(SPC):
            b = SPC * c + s
            n = int(sz[b])
            valid = (ar < n).astype(np.float32)
            m = msk[b] * valid[None, :]
            e16 = emb[b].astype(NPDT)
            e2 = (e16.astype(np.float32) ** 2).sum(1)
            x3 = np.empty((J, 128, CW), NPDT)
            x3[:, :, 0:E] = e16.reshape(J, 128, E)
            x3[:, :, E] = 1.0
            x3[:, :, E + 1] = e2.reshape(J, 128).astype(NPDT)
            ina[:, XEOFF + s * XEW : XEOFF + (s + 1) * XEW] = (
                x3.transpose(1, 0, 2).reshape(128, XEW)
            )
            m8 = m.astype(NPF8)
            mn8[:, s * MNW8 : (s + 1) * MNW8] = (
                m8.reshape(J, 128, K).transpose(1, 0, 2).reshape(128, MNW8)
            )
            mtt[s * K : (s + 1) * K, :] = m8.T
            cst[s * K : (s + 1) * K, 0] = valid
            cst[s * K : (s + 1) * K, 1] = -2.0 * valid
            pv = np.outer(valid, valid) * (1.0 - eye)
            cst[s * K : (s + 1) * K, 4 : 4 + K] = 100.0 * (1.0 - pv)
            meta.append((float(np.float64(m).sum()), n))
        ina[:, 0:XEOFF] = mn8.view(NPDT)
        in_maps.append({"cst": cst, "cs2": cs2, "ina": ina, "mtt": mtt})
    return in_maps, meta


def combine_outputs(results, meta):
    lv, ld, lr = [], [], []
    for c in range(NCORES):
        o = np.asarray(results[c]["out"], dtype=np.float64)
        for s in range(SPC):
            denom, n = meta[c * SPC + s]
            sv = o[0, 32 * s : 32 * s + 32].sum() + o[1, 32 * s : 32 * s + 32].sum()
            hh = o[s, 64:128].sum()
            rr = o[s, 128]
            lv.append(sv / denom)
            ld.append(hh / (n * (n - 1)) if n > 1 else 0.0)
            lr.append(rr / n)
    loss = np.mean(lv) + np.mean(ld) + 0.001 * np.mean(lr)
    return np.float32(loss)


def kernel(embedded, masks, size):
    nc = _build_nc()
    in_maps, meta = pack_inputs(embedded, masks, size)
    res = run_bass_kernel_spmd(nc, in_maps, core_ids=list(range(NCORES)))
    return combine_outputs(res.results, meta)


# revision 20
# speedup vs baseline: 1.0118x; 1.0118x over previous
"""Trainium2 Bass kernel for nn_DiscriminativeLoss.

Shapes (hardcoded): embedded [16, 4096, 32] f32, masks [16, 4096, 64] f32,
size [16] i32.  Data-parallel over batch: 2 samples per NeuronCore x 8 cores.

Per-sample math (fp8 masks x fp16 operands, fp32 PSUM accumulation):
  MM-A   SUMS[k, 0:33]  = sum_n m[n,k] * [e | 1][n, :]     (centroid sums+counts)
  W  = [-2c | c2 | 1],  W2 = [c | 1 | c2]  where c = valid * sums / max(cnt,1)
  MM-B   CSEL[n, :] = m[n, :] @ W                           (per-point gather)
  d2o[n] = sum_j X[n,j]*CSEL[n,j],  X = [e | 1 | e2]        (= ||e_n - c_own||^2)
  jv2[n] = relu(sqrt(d2o) - 0.5)^2                          (L_v numerator terms)
  D2P    = T(W2)^T @ T(W) = -2 c.c' + c2[k] + c2[k']        (pair distances)
  hd2    = relu(3 - sqrt(max(D2P, 0) + pvbig))^2            (L_d numerator terms)
  rtv    = valid * sqrt(c2)                                 (L_r numerator terms)
A final ones-matmul reduces all numerator terms over partitions to [2, 132];
host does the remaining tiny column sums, denominators and the batch mean.

Masks ship as fp8 (0/1 exact) in two layouts (natural for MM-A stationary,
transposed for MM-B stationary w/ fast-weight-load), packed into one fp16
input tensor read back via a bitcast view.  A dummy sqrt at kernel start
prefetches the single ACT table set during the input-DMA window; ACT
evacuates MM-B's PSUM blocks to fp16 so the big elementwise multiplies run
in the DVE 2x packed mode, and the final partition reduction is one
ones-matmul whose [2, 132] output the host finishes summing.  Relies on
masks rows being one-hot (exactly what reference.setup_inputs produces).
"""

import numpy as np
import ml_dtypes

import concourse.bacc as bacc
import concourse.mybir as mybir
from concourse import tile
from concourse.bass_utils import run_bass_kernel_spmd
from concourse.mybir import ActivationFunctionType as Act, AluOpType as Op

B, N, K, E = 16, 4096, 64, 32
NCORES = 8
SPC = B // NCORES          # samples per core
J = N // 128               # 32 n-chunks of 128
CW = E + 2                 # 34: [e | 1 | e2]
DT = mybir.dt.float16
F8 = mybir.dt.float8e4
F32 = mybir.dt.float32
NPDT = np.float16
NPF8 = ml_dtypes.float8_e4m3

XEW = J * CW               # 1088 fp16 cols per sample of [e|1|e2]
MNW8 = J * K               # 2048 fp8 cols per sample of mask-natural
XEOFF = MNW8               # 2048 fp16 cols hold both samples' fp8 mn blocks
INAW = XEOFF + SPC * XEW   # 4224 fp16 cols
CSTW = 72
CS2W = 68
OUTW = 132

_CACHE = {}


def _build_nc():
    if "nc" in _CACHE:
        return _CACHE["nc"]
    nc = bacc.Bacc("TRN2", target_bir_lowering=False, debug=False)
    cst_d = nc.dram_tensor("cst", [128, CSTW], F32, kind="ExternalInput").ap()
    cs2_d = nc.dram_tensor("cs2", [128, CS2W], DT, kind="ExternalInput").ap()
    ina_d = nc.dram_tensor("ina", [128, INAW], DT, kind="ExternalInput").ap()
    mtt_d = nc.dram_tensor("mtt", [128, N], F8, kind="ExternalInput").ap()
    out_d = nc.dram_tensor("out", [2, OUTW], F32, kind="ExternalOutput").ap()

    # ---- SBUF homes; all input DMAs are issued in-context on the Sync
    # ring, which drains FIFO at ~190GB/s: chunk-half A of the inputs
    # lands first so MM-A's first 16 chunks overlap half B's transfer,
    # and the mtt halves trail so each arrives just before its MM-B half.
    CST = nc.alloc_sbuf_tensor("cst_sb", [128, CSTW], F32).ap()
    CS2 = nc.alloc_sbuf_tensor("cs2_sb", [128, CS2W], DT).ap()
    INA = nc.alloc_sbuf_tensor("ina_sb", [128, INAW], DT).ap()
    MTT = nc.alloc_sbuf_tensor("mtt_sb", [128, N], F8).ap()

    INAF8 = INA.bitcast(F8)
    HB = INAW // 2             # 2112: fp16 cols per chunk-half block

    def mn(s, j):              # mask-natural chunk j of sample s  [128, 64] fp8
        h, jj = j // 16, j % 16
        base = 2 * HB * h + 1024 * s + K * jj
        return INAF8[:, base : base + K]

    def xec(s, j):             # [e|1] cols of chunk j for MM-A  (fp16)
        h, jj = j // 16, j % 16
        base = HB * h + 1024 + 544 * s + CW * jj
        return INA[:, base : base + 33]

    def xet(s, h):             # [e|1|e2] block of h-half for the tail (fp16)
        base = HB * h + 1024 + 544 * s
        return INA[:, base : base + 544]

    valid_c = CST[:, 0:1]
    vm2_c = CST[:, 1:2]        # -2 * valid
    b3_c = CST[:, 2:3]         # 3.0
    pvbig_c = CST[:, 4 : 4 + K]
    IDN = CS2[:, 0:64]
    ONES2 = CS2[:, 64:66]      # col0 = lower-half ones, col1 = upper-half ones

    with tile.TileContext(nc) as tc:
        with (
            tc.tile_pool(name="io", bufs=1) as io,
            tc.tile_pool(name="wk", bufs=2) as wk,
            tc.tile_pool(name="ps", bufs=1, space="PSUM") as ps,
        ):
            # input DMAs, Sync-ring FIFO order: inputs half A, consts,
            # half B, then the two mtt halves.
            nc.sync.dma_start(INA[:, 0:HB], ina_d[:, 0:HB])
            nc.sync.dma_start(CST[:], cst_d[:])
            nc.sync.dma_start(CS2[:], cs2_d[:])
            nc.sync.dma_start(INA[:, HB:INAW], ina_d[:, HB:INAW])
            nc.sync.dma_start(MTT[:, 0:2048], mtt_d[:, 0:2048])
            nc.sync.dma_start(MTT[:, 2048:N], mtt_d[:, 2048:N])

            # dummy sqrt: triggers the single ACT table-set load (~1.3us)
            # during the input-DMA window instead of mid-kernel.
            warm_i = wk.tile([128, 1], F32, tag="warm_i")
            warm_o = wk.tile([128, 1], F32, tag="warm_o")
            nc.gpsimd.memset(warm_i[:], 4.0)
            nc.scalar.activation(warm_o[:], warm_i[:], Act.Sqrt)

            WST = wk.tile([128, CW], DT, tag="wst")    # [-2c | c2 | 1]
            W2 = wk.tile([128, CW], DT, tag="w2")      # [c | 1 | c2]
            FINSRC = wk.tile([128, OUTW], DT, tag="finsrc")
            nc.vector.memset(WST[:, 33:34], 1.0)
            nc.vector.memset(W2[:, 32:33], 1.0)
            nc.vector.memset(FINSRC[:, 129:132], 0.0)

            # ---- MM-A: both samples concurrently via column tiling ----
            SUMS = ps.tile([128, 64], F32, tag="sumsa")
            SPS = [SUMS[0:K], SUMS[K:128]]
            for j in range(J):
                for s in range(SPC):
                    nc.tensor.matmul(
                        SPS[s][:, 0:33], mn(s, j), xec(s, j),
                        start=(j == 0), stop=(j == J - 1),
                        tile_position=(0, 64 * s),
                    )

            # ---- centroid factors, both samples at once ----
            D2O = wk.tile([128, 66], F32, tag="d2o")   # cols 0:64 d2o, 64 = c2
            cnt1 = wk.tile([128, 1], F32, tag="cnt1")
            nc.vector.tensor_scalar(cnt1[:], SUMS[:, 32:33], 1.0, None, Op.max)
            rec = wk.tile([128, 1], F32, tag="rec")
            nc.vector.reciprocal(rec[:], cnt1[:])
            recm2 = wk.tile([128, 1], F32, tag="recm2")
            nc.vector.tensor_scalar(recm2[:], rec[:], vm2_c, None, Op.mult)
            # c2 = 0.25 * recm2^2 * sum(sums^2)
            ssq = wk.tile([128, 1], F32, tag="ssq")
            sqj = wk.tile([128, 32], F32, tag="sqj")
            nc.scalar.activation(
                sqj[:], SUMS[:, 0:32], Act.Square, accum_out=ssq[:]
            )
            nc.scalar.activation(
                WST[:, 0:32], SUMS[:, 0:32], Act.Copy, bias=0.0, scale=recm2[:]
            )
            r2 = wk.tile([128, 1], F32, tag="r2")
            nc.vector.tensor_tensor(r2[:], recm2[:], recm2[:], Op.mult)
            nc.vector.scalar_tensor_tensor(
                D2O[:, 64:65], ssq[:], 0.25, r2[:], Op.mult, Op.mult
            )
            nc.vector.tensor_copy(WST[:, 32:33], D2O[:, 64:65])
            recp1 = wk.tile([128, 1], F32, tag="recp1")
            nc.vector.tensor_scalar(recp1[:], rec[:], valid_c, None, Op.mult)
            nc.scalar.activation(
                W2[:, 0:32], SUMS[:, 0:32], Act.Copy, bias=0.0, scale=recp1[:]
            )
            nc.vector.tensor_copy(W2[:, 33:34], D2O[:, 64:65])

            # ---- MM-B + per-point distances; samples on row-groups ----
            PBS = [None, None]
            for h in range(2):
                for s in range(SPC):
                    PB = ps.tile([128, 1024], F32, tag=f"pb{s}")
                    PBS[s] = PB
                    for i in range(16):
                        j = h * 16 + i
                        off = 512 * (i // 8) + CW * (i % 8)
                        nc.tensor.matmul(
                            PB[:, off : off + CW],
                            MTT[s * K : (s + 1) * K, j * 128 : (j + 1) * 128],
                            WST[s * K : (s + 1) * K, 0:CW],
                            start=True, stop=True,
                            tile_position=(64 * s, 0),
                        )
                for s in range(SPC):
                    PB = PBS[s]
                    EV = wk.tile([128, 2 * 8 * CW], DT, tag=f"ev{s}")
                    pb3 = PB[:].rearrange("p (b q) -> p b q", b=2)[:, :, 0 : 8 * CW]
                    ev3 = EV[:].rearrange("p (b q) -> p b q", b=2)
                    nc.scalar.activation(ev3, pb3, Act.Copy)
                    PR = wk.tile([128, 2 * 8 * CW], DT, tag="pr")
                    pr3 = PR[:].rearrange("p (b q) -> p b q", b=2)
                    nc.vector.tensor_tensor(pr3, EV[:], xet(s, h), Op.mult)
                    nc.vector.tensor_reduce(
                        D2O[:, s * 32 + h * 16 : s * 32 + h * 16 + 16],
                        PR[:].rearrange("p (j c) -> p j c", c=CW),
                        axis=mybir.AxisListType.X,
                        op=Op.add,
                    )

            # ---- L_d: pair distances from transposed W / W2 ----
            TWt = ps.tile([128, K], DT, tag="twt")
            LTt = ps.tile([128, K], DT, tag="ltt")
            for s in range(SPC):
                nc.tensor.transpose(
                    TWt[64 * s : 64 * s + CW, :],
                    WST[s * K : (s + 1) * K, 0:CW],
                    IDN[s * K : (s + 1) * K, :],
                    tile_position=(64 * s, 64 * s),
                )
                nc.tensor.transpose(
                    LTt[64 * s : 64 * s + CW, :],
                    W2[s * K : (s + 1) * K, 0:CW],
                    IDN[s * K : (s + 1) * K, :],
                    tile_position=(64 * s, 64 * s),
                )
            TW = wk.tile([128, K], DT, tag="tw")
            LT = wk.tile([128, K], DT, tag="lt")
            nc.scalar.activation(TW[:], TWt[:], Act.Copy)
            nc.scalar.activation(LT[:], LTt[:], Act.Copy)
            D2P = ps.tile([128, K], F32, tag="sumsa")
            for s in range(SPC):
                nc.tensor.matmul(
                    D2P[64 * s : 64 * s + 64, :],
                    LT[64 * s : 64 * s + CW, :],
                    TW[64 * s : 64 * s + CW, :],
                    start=True, stop=True,
                    tile_position=(64 * s, 64 * s),
                )
            DSm = wk.tile([128, K], F32, tag="dsm")
            nc.vector.scalar_tensor_tensor(
                DSm[:], D2P[:], 0.0, pvbig_c, Op.max, Op.add
            )
            NS = wk.tile([128, K], F32, tag="ns")
            nc.scalar.activation(NS[:], DSm[:], Act.Sqrt)
            HD = wk.tile([128, K], F32, tag="hd")
            nc.scalar.activation(HD[:], NS[:], Act.Relu, bias=b3_c, scale=-1.0)
            nc.gpsimd.tensor_tensor(FINSRC[:, 64:128], HD[:], HD[:], Op.mult)

            # ---- L_v + L_r tails ----
            DN65 = wk.tile([128, 65], F32, tag="dn65")
            nc.scalar.activation(DN65[:], D2O[:, 0:65], Act.Sqrt)
            HV = wk.tile([128, 64], DT, tag="hv")
            nc.vector.tensor_scalar(HV[:], DN65[:, 0:64], -0.5, 0.0, Op.add, Op.max)
            nc.vector.tensor_tensor(FINSRC[:, 0:64], HV[:], HV[:], Op.mult)
            nc.vector.tensor_scalar(
                FINSRC[:, 128:129], DN65[:, 64:65], valid_c, None, Op.mult
            )

            # ---- partition reduction: row 0 = lower half, row 1 = upper ----
            FIN = ps.tile([2, OUTW], F32, tag="twt")
            nc.tensor.matmul(FIN[:], ONES2, FINSRC[:], start=True, stop=True)
            FOUT = wk.tile([2, OUTW], F32, tag="fout")
            nc.vector.tensor_copy(FOUT[:], FIN[:])
            nc.sync.dma_start(out_d[:], FOUT[:])

    nc.compile()
    _CACHE["nc"] = nc
    return nc


def pack_inputs(embedded, masks, size):
    emb = np.asarray(embedded, dtype=np.float32)
    msk = np.asarray(masks, dtype=np.float32)
    sz = np.asarray(size).astype(np.int64)
    ar = np.arange(K)
    eye = np.eye(K, dtype=np.float32)
    in_maps, meta = [], []
    for c in range(NCORES):
        ina = np.zeros((128, INAW), NPDT)
        mtt = np.zeros((128, N), NPF8)
        cst = np.zeros((128, CSTW), np.float32)
        cs2 = np.zeros((128, CS2W), NPDT)
        cst[:, 2] = 3.0
        cs2[0:K, 0:K] = eye
        cs2[K:128, 0:K] = eye
        cs2[0:K, 64] = 1.0
        cs2[K:128, 65] = 1.0
        for s in range(SPC):
            b = SPC * c + s
            n = int(sz[b])
            valid = (ar < n).astype(np.float32)
            m = msk[b] * valid[None, :]
            e16 = emb[b].astype(NPDT)
            e2 = (e16.astype(np.float32) ** 2).sum(1)
            x3 = np.empty((J, 128, CW), NPDT)
            x3[:, :, 0:E] = e16.reshape(J, 128, E)
            x3[:, :, E] = 1.0
            x3[:, :, E + 1] = e2.reshape(J, 128).astype(NPDT)
            xs = x3.transpose(1, 0, 2).reshape(128, XEW)
            m8 = m.astype(NPF8)
            mns = m8.reshape(J, 128, K).transpose(1, 0, 2).reshape(128, MNW8)
            HB = INAW // 2
            for h in range(2):
                ina[:, HB * h + 512 * s : HB * h + 512 * (s + 1)] = (
                    mns[:, h * 1024 : (h + 1) * 1024].view(NPDT)
                )
                ina[:, HB * h + 1024 + 544 * s : HB * h + 1024 + 544 * (s + 1)] = (
                    xs[:, h * 544 : (h + 1) * 544]
                )
            mtt[s * K : (s + 1) * K, :] = m8.T
            cst[s * K : (s + 1) * K, 0] = valid
            cst[s * K : (s + 1) * K, 1] = -2.0 * valid
            pv = np.outer(valid, valid) * (1.0 - eye)
            cst[s * K : (s + 1) * K, 4 : 4 + K] = 100.0 * (1.0 - pv)
            meta.append((float(np.float64(m).sum()), n))
        in_maps.append({"cst": cst, "cs2": cs2, "ina": ina, "mtt": mtt})
    return in_maps, meta


def combine_outputs(results, meta):
    lv, ld, lr = [], [], []
    for c in range(NCORES):
        o = np.asarray(results[c]["out"], dtype=np.float64)
        for s in range(SPC):
            denom, n = meta[c * SPC + s]
            sv = o[0, 32 * s : 32 * s + 32].sum() + o[1, 32 * s : 32 * s + 32].sum()
            hh = o[s, 64:128].sum()
            rr = o[s, 128]
            lv.append(sv / denom)
            ld.append(hh / (n * (n - 1)) if n > 1 else 0.0)
            lr.append(rr / n)
    loss = np.mean(lv) + np.mean(ld) + 0.001 * np.mean(lr)
    return np.float32(loss)


def kernel(embedded, masks, size):
    nc = _build_nc()
    in_maps, meta = pack_inputs(embedded, masks, size)
    res = run_bass_kernel_spmd(nc, in_maps, core_ids=list(range(NCORES)))
    return combine_outputs(res.results, meta)


# revision 22
# speedup vs baseline: 1.0282x; 1.0162x over previous
"""Trainium2 Bass kernel for nn_DiscriminativeLoss.

Shapes (hardcoded): embedded [16, 4096, 32] f32, masks [16, 4096, 64] f32,
size [16] i32.  Data-parallel over batch: 2 samples per NeuronCore x 8 cores.

Per-sample math (fp8 masks x fp16 operands, fp32 PSUM accumulation):
  MM-A   SUMS[k, 0:33]  = sum_n m[n,k] * [e | 1][n, :]     (centroid sums+counts)
  W  = [-2c | c2 | 1],  W2 = [c | 1 | c2]  where c = valid * sums / max(cnt,1)
  MM-B   CSEL[n, :] = m[n, :] @ W                           (per-point gather)
  d2o[n] = sum_j X[n,j]*CSEL[n,j],  X = [e | 1 | e2]        (= ||e_n - c_own||^2)
  jv2[n] = relu(sqrt(d2o) - 0.5)^2                          (L_v numerator terms)
  D2P    = T(W2)^T @ T(W) = -2 c.c' + c2[k] + c2[k']        (pair distances)
  hd2    = relu(3 - sqrt(max(D2P, 0) + pvbig))^2            (L_d numerator terms)
  rtv    = valid * sqrt(c2)                                 (L_r numerator terms)
A final ones-matmul reduces all numerator terms over partitions to [2, 132];
host does the remaining tiny column sums, denominators and the batch mean.

Masks ship as fp8 (0/1 exact) in two layouts (natural for MM-A stationary,
transposed for MM-B stationary w/ fast-weight-load), packed into one fp16
input tensor read back via a bitcast view.  A dummy sqrt at kernel start
prefetches the single ACT table set during the input-DMA window; ACT
evacuates MM-B's PSUM blocks to fp16 so the big elementwise multiplies run
in the DVE 2x packed mode, and the final partition reduction is one
ones-matmul whose [2, 132] output the host finishes summing.  Relies on
masks rows being one-hot (exactly what reference.setup_inputs produces).
"""

import numpy as np
import ml_dtypes

import concourse.bacc as bacc
import concourse.mybir as mybir
from concourse import tile
from concourse.bass_utils import run_bass_kernel_spmd
from concourse.mybir import ActivationFunctionType as Act, AluOpType as Op

B, N, K, E = 16, 4096, 64, 32
NCORES = 8
SPC = B // NCORES          # samples per core
J = N // 128               # 32 n-chunks of 128
CW = E + 2                 # 34: [e | 1 | e2]
DT = mybir.dt.float16
F8 = mybir.dt.float8e4
F32 = mybir.dt.float32
NPDT = np.float16
NPF8 = ml_dtypes.float8_e4m3

XEW = J * CW               # 1088 fp16 cols per sample of [e|1|e2]
MNW8 = J * K               # 2048 fp8 cols per sample of mask-natural
XEOFF = MNW8               # 2048 fp16 cols hold both samples' fp8 mn blocks
INAW = XEOFF + SPC * XEW   # 4224 fp16 cols
CSTW = 72
CS2W = 68
OUTW = 132

_CACHE = {}


def _build_nc():
    if "nc" in _CACHE:
        return _CACHE["nc"]
    nc = bacc.Bacc("TRN2", target_bir_lowering=False, debug=False)
    cst_d = nc.dram_tensor("cst", [128, CSTW], F32, kind="ExternalInput").ap()
    cs2_d = nc.dram_tensor("cs2", [128, CS2W], DT, kind="ExternalInput").ap()
    ina_d = nc.dram_tensor("ina", [128, INAW], DT, kind="ExternalInput").ap()
    mtt_d = nc.dram_tensor("mtt", [128, N], F8, kind="ExternalInput").ap()
    out_d = nc.dram_tensor("out", [2, OUTW], F32, kind="ExternalOutput").ap()

    # ---- SBUF homes; all input DMAs are issued in-context on the Sync
    # ring, which drains FIFO at ~190GB/s: chunk-half A of the inputs
    # lands first so MM-A's first 16 chunks overlap half B's transfer,
    # and the mtt halves trail so each arrives just before its MM-B half.
    CST = nc.alloc_sbuf_tensor("cst_sb", [128, CSTW], F32).ap()
    CS2 = nc.alloc_sbuf_tensor("cs2_sb", [128, CS2W], DT).ap()
    INA = nc.alloc_sbuf_tensor("ina_sb", [128, INAW], DT).ap()
    MTT = nc.alloc_sbuf_tensor("mtt_sb", [128, N], F8).ap()

    INAF8 = INA.bitcast(F8)
    HB = INAW // 2             # 2112: fp16 cols per chunk-half block

    def mn(s, j):              # mask-natural chunk j of sample s  [128, 64] fp8
        h, jj = j // 16, j % 16
        base = 2 * HB * h + 1024 * s + K * jj
        return INAF8[:, base : base + K]

    def xec(s, j):             # [e|1] cols of chunk j for MM-A  (fp16)
        h, jj = j // 16, j % 16
        base = HB * h + 1024 + 544 * s + CW * jj
        return INA[:, base : base + 33]

    def xet(s, h):             # [e|1|e2] block of h-half for the tail (fp16)
        base = HB * h + 1024 + 544 * s
        return INA[:, base : base + 544]

    valid_c = CST[:, 0:1]
    vm2_c = CST[:, 1:2]        # -2 * valid
    b3_c = CST[:, 2:3]         # 3.0
    pvbig_c = CST[:, 4 : 4 + K]
    IDN = CS2[:, 0:64]
    ONES2 = CS2[:, 64:66]      # col0 = lower-half ones, col1 = upper-half ones

    with tile.TileContext(nc) as tc:
        with (
            tc.tile_pool(name="io", bufs=1) as io,
            tc.tile_pool(name="wk", bufs=2) as wk,
            tc.tile_pool(name="ps", bufs=1, space="PSUM") as ps,
        ):
            # input DMAs, Sync-ring FIFO order: inputs half A, consts,
            # half B, then the two mtt halves.
            nc.sync.dma_start(INA[:, 0:HB], ina_d[:, 0:HB])
            nc.sync.dma_start(CST[:], cst_d[:])
            nc.sync.dma_start(CS2[:], cs2_d[:])
            nc.sync.dma_start(INA[:, HB:INAW], ina_d[:, HB:INAW])
            nc.sync.dma_start(MTT[:, 0:2048], mtt_d[:, 0:2048])
            nc.sync.dma_start(MTT[:, 2048:N], mtt_d[:, 2048:N])

            # dummy sqrt: triggers the single ACT table-set load (~1.3us)
            # during the input-DMA window instead of mid-kernel.
            warm_i = wk.tile([128, 1], F32, tag="warm_i")
            warm_o = wk.tile([128, 1], F32, tag="warm_o")
            nc.gpsimd.memset(warm_i[:], 4.0)
            nc.scalar.activation(warm_o[:], warm_i[:], Act.Sqrt)

            WST = wk.tile([128, CW], DT, tag="wst")    # [-2c | c2 | 1]
            W2 = wk.tile([128, CW], DT, tag="w2")      # [c | 1 | c2]
            FINSRC = wk.tile([128, OUTW], DT, tag="finsrc")
            nc.vector.memset(WST[:, 33:34], 1.0)
            nc.vector.memset(W2[:, 32:33], 1.0)
            nc.vector.memset(FINSRC[:, 129:132], 0.0)

            # ---- MM-A: both samples concurrently via column tiling ----
            SUMS = ps.tile([128, 64], F32, tag="sumsa")
            SPS = [SUMS[0:K], SUMS[K:128]]
            for j in range(J):
                for s in range(SPC):
                    nc.tensor.matmul(
                        SPS[s][:, 0:33], mn(s, j), xec(s, j),
                        start=(j == 0), stop=(j == J - 1),
                        tile_position=(0, 64 * s),
                    )

            # ---- centroid factors, both samples at once ----
            D2O = wk.tile([128, 66], F32, tag="d2o")   # cols 0:64 d2o, 64 = c2
            cnt1 = wk.tile([128, 1], F32, tag="cnt1")
            nc.vector.tensor_scalar(cnt1[:], SUMS[:, 32:33], 1.0, None, Op.max)
            rec = wk.tile([128, 1], F32, tag="rec")
            nc.vector.reciprocal(rec[:], cnt1[:])
            recm2 = wk.tile([128, 1], F32, tag="recm2")
            nc.vector.tensor_scalar(recm2[:], rec[:], vm2_c, None, Op.mult)
            # c2 = 0.25 * recm2^2 * sum(sums^2)
            ssq = wk.tile([128, 1], F32, tag="ssq")
            sqj = wk.tile([128, 32], F32, tag="sqj")
            nc.scalar.activation(
                sqj[:], SUMS[:, 0:32], Act.Square, accum_out=ssq[:]
            )
            nc.scalar.activation(
                WST[:, 0:32], SUMS[:, 0:32], Act.Copy, bias=0.0, scale=recm2[:]
            )
            r2 = wk.tile([128, 1], F32, tag="r2")
            nc.vector.tensor_tensor(r2[:], recm2[:], recm2[:], Op.mult)
            nc.vector.scalar_tensor_tensor(
                D2O[:, 64:65], ssq[:], 0.25, r2[:], Op.mult, Op.mult
            )
            nc.vector.tensor_copy(WST[:, 32:33], D2O[:, 64:65])
            recp1 = wk.tile([128, 1], F32, tag="recp1")
            nc.vector.tensor_scalar(recp1[:], rec[:], valid_c, None, Op.mult)
            nc.scalar.activation(
                W2[:, 0:32], SUMS[:, 0:32], Act.Copy, bias=0.0, scale=recp1[:]
            )
            nc.vector.tensor_copy(W2[:, 33:34], D2O[:, 64:65])

            # ---- MM-B + per-point distances; samples on row-groups ----
            PBS = [None, None]
            for h in range(2):
                for s in range(SPC):
                    PB = ps.tile([128, 1024], F32, tag=f"pb{s}")
                    PBS[s] = PB
                    for i in range(16):
                        j = h * 16 + i
                        off = 512 * (i // 8) + CW * (i % 8)
                        nc.tensor.matmul(
                            PB[:, off : off + CW],
                            MTT[s * K : (s + 1) * K, j * 128 : (j + 1) * 128],
                            WST[s * K : (s + 1) * K, 0:CW],
                            start=True, stop=True,
                            tile_position=(64 * s, 0),
                        )
                for s in range(SPC):
                    PB = PBS[s]
                    EV = wk.tile([128, 2 * 8 * CW], DT, tag=f"ev{s}")
                    pb3 = PB[:].rearrange("p (b q) -> p b q", b=2)[:, :, 0 : 8 * CW]
                    ev3 = EV[:].rearrange("p (b q) -> p b q", b=2)
                    nc.scalar.activation(ev3, pb3, Act.Copy)
                    PR = wk.tile([128, 2 * 8 * CW], DT, tag="pr")
                    pr3 = PR[:].rearrange("p (b q) -> p b q", b=2)
                    nc.vector.tensor_tensor(pr3, EV[:], xet(s, h), Op.mult)
                    nc.vector.tensor_reduce(
                        D2O[:, s * 32 + h * 16 : s * 32 + h * 16 + 16],
                        PR[:].rearrange("p (j c) -> p j c", c=CW),
                        axis=mybir.AxisListType.X,
                        op=Op.add,
                    )

            # ---- L_d: pair distances from transposed W / W2 ----
            TWt = ps.tile([128, K], DT, tag="twt")
            LTt = ps.tile([128, K], DT, tag="ltt")
            for s in range(SPC):
                nc.tensor.transpose(
                    TWt[64 * s : 64 * s + CW, :],
                    WST[s * K : (s + 1) * K, 0:CW],
                    IDN[s * K : (s + 1) * K, :],
                    tile_position=(64 * s, 64 * s),
                )
                nc.tensor.transpose(
                    LTt[64 * s : 64 * s + CW, :],
                    W2[s * K : (s + 1) * K, 0:CW],
                    IDN[s * K : (s + 1) * K, :],
                    tile_position=(64 * s, 64 * s),
                )
            TW = wk.tile([128, K], DT, tag="tw")
            LT = wk.tile([128, K], DT, tag="lt")
            nc.scalar.activation(TW[:], TWt[:], Act.Copy)
            nc.scalar.activation(LT[:], LTt[:], Act.Copy)
            D2P = ps.tile([128, K], F32, tag="sumsa")
            for s in range(SPC):
                nc.tensor.matmul(
                    D2P[64 * s : 64 * s + 64, :],
                    LT[64 * s : 64 * s + CW, :],
                    TW[64 * s : 64 * s + CW, :],
                    start=True, stop=True,
                    tile_position=(64 * s, 64 * s),
                )
            DSm = wk.tile([128, K], F32, tag="dsm")
            nc.vector.scalar_tensor_tensor(
                DSm[:], D2P[:], 0.0, pvbig_c, Op.max, Op.add
            )
            NS = wk.tile([128, K], F32, tag="ns")
            nc.scalar.activation(NS[:], DSm[:], Act.Sqrt)
            HD = wk.tile([128, K], F32, tag="hd")
            nc.scalar.activation(HD[:], NS[:], Act.Relu, bias=b3_c, scale=-1.0)
            nc.gpsimd.tensor_tensor(FINSRC[:, 64:128], HD[:], HD[:], Op.mult)

            # ---- L_v + L_r tails ----
            DN65 = wk.tile([128, 65], F32, tag="dn65")
            nc.scalar.activation(DN65[:], D2O[:, 0:65], Act.Sqrt)
            HV = wk.tile([128, 64], DT, tag="hv")
            nc.vector.tensor_scalar(HV[:], DN65[:, 0:64], -0.5, 0.0, Op.add, Op.max)
            nc.vector.tensor_tensor(FINSRC[:, 0:64], HV[:], HV[:], Op.mult)
            nc.vector.tensor_scalar(
                FINSRC[:, 128:129], DN65[:, 64:65], valid_c, None, Op.mult
            )

            # ---- partition reduction: row 0 = lower half, row 1 = upper ----
            FIN = ps.tile([2, OUTW], F32, tag="twt")
            nc.tensor.matmul(FIN[:], ONES2, FINSRC[:], start=True, stop=True)
            FOUT = wk.tile([2, OUTW], F32, tag="fout")
            nc.vector.tensor_copy(FOUT[:], FIN[:])
            nc.sync.dma_start(out_d[:], FOUT[:])

    nc.compile()
    _CACHE["nc"] = nc
    return nc


def pack_inputs(embedded, masks, size):
    emb = np.asarray(embedded, dtype=np.float32)
    msk = np.asarray(masks, dtype=np.float32)
    sz = np.asarray(size).astype(np.int64)
    ar = np.arange(K)
    eye = np.eye(K, dtype=np.float32)
    in_maps, meta = [], []
    for c in range(NCORES):
        ina = np.zeros((128, INAW), NPDT)
        mtt = np.zeros((128, N), NPF8)
        cst = np.zeros((128, CSTW), np.float32)
        cs2 = np.zeros((128, CS2W), NPDT)
        cst[:, 2] = 3.0
        cs2[0:K, 0:K] = eye
        cs2[K:128, 0:K] = eye
        cs2[0:K, 64] = 1.0
        cs2[K:128, 65] = 1.0
        for s in range(SPC):
            b = SPC * c + s
            n = int(sz[b])
            valid = (ar < n).astype(np.float32)
            m = msk[b] * valid[None, :]
            e16 = emb[b].astype(NPDT)
            e2 = (e16.astype(np.float32) ** 2).sum(1)
            x3 = np.empty((J, 128, CW), NPDT)
            x3[:, :, 0:E] = e16.reshape(J, 128, E)
            x3[:, :, E] = 1.0
            x3[:, :, E + 1] = e2.reshape(J, 128).astype(NPDT)
            xs = x3.transpose(1, 0, 2).reshape(128, XEW)
            m8 = m.astype(NPF8)
            mns = m8.reshape(J, 128, K).transpose(1, 0, 2).reshape(128, MNW8)
            HB = INAW // 2
            for h in range(2):
                ina[:, HB * h + 512 * s : HB * h + 512 * (s + 1)] = (
                    mns[:, h * 1024 : (h + 1) * 1024].view(NPDT)
                )
                ina[:, HB * h + 1024 + 544 * s : HB * h + 1024 + 544 * (s + 1)] = (
                    xs[:, h * 544 : (h + 1) * 544]
                )
            mtt[s * K : (s + 1) * K, :] = m8.T
            cst[s * K : (s + 1) * K, 0] = valid
            cst[s * K : (s + 1) * K, 1] = -2.0 * valid
            pv = np.outer(valid, valid) * (1.0 - eye)
            cst[s * K : (s + 1) * K, 4 : 4 + K] = 100.0 * (1.0 - pv)
            meta.append((float(np.float64(m).sum()), n))
        in_maps.append({"cst": cst, "cs2": cs2, "ina": ina, "mtt": mtt})
    return in_maps, meta


def combine_outputs(results, meta):
    lv, ld, lr = [], [], []
    for c in range(NCORES):
        o = np.asarray(results[c]["out"], dtype=np.float64)
        for s in range(SPC):
            denom, n = meta[c * SPC + s]
            sv = o[0, 32 * s : 32 * s + 32].sum() + o[1, 32 * s : 32 * s + 32].sum()
            hh = o[s, 64:128].sum()
            rr = o[s, 128]
            lv.append(sv / denom)
            ld.append(hh / (n * (n - 1)) if n > 1 else 0.0)
            lr.append(rr / n)
    loss = np.mean(lv) + np.mean(ld) + 0.001 * np.mean(lr)
    return np.float32(loss)


def kernel(embedded, masks, size):
    nc = _build_nc()
    in_maps, meta = pack_inputs(embedded, masks, size)
    res = run_bass_kernel_spmd(nc, in_maps, core_ids=list(range(NCORES)))
    return combine_outputs(res.results, meta)
